# revision 1
# baseline (speedup 1.0000x reference)
"""Deformable-DETR multi-scale attention kernel for 8 Trainium2 NeuronCores.

Sharding: core c handles batch b=c//2, query half (c%2)*4096..+4096.
Per-core pipeline:
  S1 value projection (PE) -> value_sb bf16 [tok%128, (tok//128)*256 + dim]
  S2 build 4-corner "patch" tables in HBM: row (t,h) = [hd, corner] bf16
     via PE partition-rotations + strided DVE copies + DMA
  S3 per 128-query chunk: off/attn projections, softmax, bilinear weights
     + table indices, indirect-DMA gather, weighted reduce (DVE)
  S4 out projection (PE) -> out rows
Host: transpose/shard inputs, assemble output.
"""

import numpy as np
import ml_dtypes

import concourse.bass as bass
import concourse.mybir as mybir
import concourse.tile as tile
from concourse import bacc
from concourse.bass import IndirectOffsetOnAxis
from concourse.bass_utils import run_bass_kernel_spmd

F32 = mybir.dt.float32
BF16 = mybir.dt.bfloat16
I32 = mybir.dt.int32
AF = mybir.ActivationFunctionType
ALU = mybir.AluOpType
AX = mybir.AxisListType

# Problem constants
N, LQ, DM, MH, NL, NP = 4, 8192, 256, 8, 4, 4
HD = DM // MH  # 32
SHAPES = [(128, 128), (64, 64), (32, 32), (16, 16)]
S = sum(h * w for h, w in SHAPES)  # 21760
LS = [0, 16384, 20480, 21504]  # level starts in value
N_CORES = 8
QPC = LQ // 2  # queries per core = 4096
NCHUNK = QPC // 128  # 32

# Table geometry: per level, entry t <-> pixel p = t - OFF_L[l]
# patch(t) = value rows [p, p+1, p+W, p+W+1], stored as a row of
# (h, hd, corner)-ordered bf16: 1024 els = 2KB per row, 128 els per head.
OFF_L = [256, 128, 128, 128]  # multiples of 128, >= W+1
TROWS_L = [16640, 4224, 1152, 384]  # HW + OFF_L, multiples of 128
TB_L = [0, 16640, 20864, 22016]
TOT_ROWS = 22400
ROW_ELS = MH * HD * 4  # 1024
# value_sb covers tok in [-256, S+256): 174 tiles of 128
VPAD = 256
VTILES = (S + 2 * VPAD) // 128  # 174

SHIFTS = [1, 16, 17, 32, 33, 64, 65]
DEBUG = False


def apv(src, extra_offset, dims):
    """Strided free-dim view: keep partition dim, replace free dims."""
    a = src if isinstance(src, bass.AP) else src[:]
    return bass.AP(a.tensor, a.offset + extra_offset, [a.ap[0]] + list(dims))


def build_nc():
    nc = bacc.Bacc("TRN2", target_bir_lowering=False, debug=False,
                   num_devices=N_CORES)

    # ---- dram I/O ----
    qT = nc.dram_tensor("qT", [DM, QPC], F32, kind="ExternalInput")
    xT = nc.dram_tensor("xT", [DM, S], F32, kind="ExternalInput")
    refsW = nc.dram_tensor("refsW", [128, NCHUNK * 32], F32, kind="ExternalInput")
    Woff = nc.dram_tensor("Woff", [DM + 1, 256], F32, kind="ExternalInput")
    Wattn = nc.dram_tensor("Wattn", [DM + 1, 128], F32, kind="ExternalInput")
    Wval = nc.dram_tensor("Wval", [DM, 256], F32, kind="ExternalInput")
    bval = nc.dram_tensor("bval", [128, 256], F32, kind="ExternalInput")
    Wout = nc.dram_tensor("Wout", [DM + 1, 256], F32, kind="ExternalInput")
    cdefs = {"cW": 4, "cWm1": 4, "cWm2": 4, "cLo": 4, "cHi": 4, "cC8h": 32}
    cdram = {k: nc.dram_tensor(k, [128, w], F32, kind="ExternalInput")
             for k, w in cdefs.items()}
    pmats = nc.dram_tensor("pmats", [len(SHIFTS) * 2 * 128, 128], BF16,
                           kind="ExternalInput")
    ident = nc.dram_tensor("ident", [128, 128], F32, kind="ExternalInput")
    out_d = nc.dram_tensor("out", [QPC, 256], F32, kind="ExternalOutput")
    if DEBUG:
        dbg_idx = nc.dram_tensor("dbg_idx", [128, 128], I32, kind="ExternalOutput")
        dbg_g = nc.dram_tensor("dbg_g", [128, 8192], BF16, kind="ExternalOutput")
        dbg_w4 = nc.dram_tensor("dbg_w4", [128, 512], BF16, kind="ExternalOutput")
        dbg_smpl = nc.dram_tensor("dbg_smpl", [128, 256], F32, kind="ExternalOutput")
        dbg_val = nc.dram_tensor("dbg_val", [128, 1024], BF16, kind="ExternalOutput")
        dbg_tbl = nc.dram_tensor("dbg_tbl", [128, ROW_ELS], BF16, kind="ExternalOutput")

    with tile.TileContext(nc) as tc:
        with (
            tc.tile_pool(name="persist", bufs=1) as pp,
            tc.tile_pool(name="dram", bufs=1, space="DRAM") as dp,
        ):
            tables = dp.tile([TOT_ROWS, ROW_ELS], BF16, tag="tables")

            # persistent SBUF tiles
            qT_sb = pp.tile([128, 2 * QPC], F32, tag="qT")
            nc.sync.dma_start(qT_sb[:, 0:QPC], qT[0:128, :])
            nc.sync.dma_start(qT_sb[:, QPC:], qT[128:256, :])
            refsW_sb = pp.tile([128, NCHUNK * 32], F32, tag="refsW")
            nc.sync.dma_start(refsW_sb[:], refsW[:])
            wv_sb = pp.tile([128, 2 * 256], F32, tag="wv")
            nc.sync.dma_start(wv_sb[:, 0:256], Wval[0:128, :])
            nc.sync.dma_start(wv_sb[:, 256:], Wval[128:256, :])
            bval_sb = pp.tile([128, 256], F32, tag="bval")
            nc.sync.dma_start(bval_sb[:], bval[:])
            wo_sb = pp.tile([128, 2 * 256], F32, tag="wo")
            nc.sync.dma_start(wo_sb[:, 0:256], Woff[0:128, :])
            nc.sync.dma_start(wo_sb[:, 256:512], Woff[128:256, :])
            wo_b = pp.tile([1, 256], F32, tag="wo_b")
            nc.sync.dma_start(wo_b[:], Woff[256:257, :])
            wa_sb = pp.tile([128, 2 * 128], F32, tag="wa")
            nc.sync.dma_start(wa_sb[:, 0:128], Wattn[0:128, :])
            nc.sync.dma_start(wa_sb[:, 128:256], Wattn[128:256, :])
            wa_b = pp.tile([1, 128], F32, tag="wa_b")
            nc.sync.dma_start(wa_b[:], Wattn[256:257, :])
            wq_sb = pp.tile([128, 2 * 256], F32, tag="wq")
            nc.sync.dma_start(wq_sb[:, 0:256], Wout[0:128, :])
            nc.sync.dma_start(wq_sb[:, 256:512], Wout[128:256, :])
            wq_b = pp.tile([1, 256], F32, tag="wq_b")
            nc.sync.dma_start(wq_b[:], Wout[256:257, :])
            consts = {}
            for name, w in cdefs.items():
                consts[name] = pp.tile([128, w], F32, tag=name, name=name + "_sb")
                nc.sync.dma_start(consts[name][:], cdram[name][:])
            pm_sb = pp.tile([128, len(SHIFTS) * 2 * 128], BF16, tag="pm")
            for i in range(len(SHIFTS) * 2):
                nc.sync.dma_start(pm_sb[:, i * 128:(i + 1) * 128],
                                  pmats[i * 128:(i + 1) * 128, :])
            id_sb = pp.tile([128, 128], F32, tag="ident")
            nc.sync.dma_start(id_sb[:], ident[:])
            ones_sb = pp.tile([1, 128], F32, tag="ones")
            nc.vector.memset(ones_sb[:], 1.0)

            def pmat(s, which):  # which: 0=P (main), 1=Q (boundary)
                i = SHIFTS.index(s) * 2 + which
                return pm_sb[:, i * 128:(i + 1) * 128]

            with tc.tile_pool(name="vsb", bufs=1) as vp:
                value_sb = vp.tile([128, VTILES * 256], BF16, tag="value")

                def vtile(k):  # value_sb tile k (tok = (k-2)*128 + part)
                    assert 0 <= k < VTILES, k
                    return value_sb[:, k * 256:(k + 1) * 256]

                # ================= S1: value projection =================
                with (
                    tc.tile_pool(name="xslab", bufs=2) as xp,
                    tc.tile_pool(name="vpsum", bufs=4, space="PSUM") as vpp,
                ):
                    nc.vector.memset(value_sb[:, 0:512], 0.0)
                    nc.vector.memset(value_sb[:, (VTILES - 2) * 256:], 0.0)
                    SLAB = 2048
                    for s0 in range(0, S, SLAB):
                        sl = min(SLAB, S - s0)
                        xt = xp.tile([128, 2 * SLAB], F32, tag="xslab")
                        nc.sync.dma_start(xt[:, 0:sl], xT[0:128, s0:s0 + sl])
                        nc.sync.dma_start(xt[:, SLAB:SLAB + sl],
                                          xT[128:256, s0:s0 + sl])
                        for t0 in range(0, sl, 128):
                            pv = vpp.tile([128, 256], F32, tag="vpsum")
                            nc.tensor.matmul(pv[:], xt[:, t0:t0 + 128],
                                             wv_sb[:, 0:256], start=True, stop=False)
                            nc.tensor.matmul(pv[:], xt[:, SLAB + t0:SLAB + t0 + 128],
                                             wv_sb[:, 256:512], start=False, stop=True)
                            vt = (s0 + t0) // 128 + 2
                            nc.vector.tensor_tensor(
                                value_sb[:, vt * 256:(vt + 1) * 256], pv[:],
                                bval_sb[:], ALU.add)

                if DEBUG:
                    nc.sync.dma_start(dbg_val[:, 0:512], value_sb[:, 2 * 256:4 * 256])
                    nc.sync.dma_start(dbg_val[:, 512:1024],
                                      value_sb[:, 130 * 256:132 * 256])
                # ================= S2: patch tables =================
                with (
                    tc.tile_pool(name="stage", bufs=3) as sp,
                    tc.tile_pool(name="rpsum", bufs=6, space="PSUM") as rp,
                ):
                    def rot(s, k):
                        """rows (k*128 + part + s) of padded value."""
                        pr = rp.tile([128, 256], F32, tag="rot")
                        nc.tensor.matmul(pr[:], pmat(s, 0), vtile(k),
                                         start=True, stop=False)
                        nc.tensor.matmul(pr[:], pmat(s, 1), vtile(k + 1),
                                         start=False, stop=True)
                        return pr[:]

                    for l, (H, W) in enumerate(SHAPES):
                        ntile = TROWS_L[l] // 128
                        k0 = (LS[l] - OFF_L[l] + VPAD) // 128
                        for it in range(ntile):
                            st = sp.tile([128, ROW_ELS], BF16, tag="stage")
                            if W == 128:
                                srcs = [vtile(k0 + it), rot(1, k0 + it),
                                        vtile(k0 + it + 1), rot(1, k0 + it + 1)]
                            else:
                                srcs = [vtile(k0 + it), rot(1, k0 + it),
                                        rot(W, k0 + it), rot(W + 1, k0 + it)]
                            for ci, src in enumerate(srcs):
                                nc.vector.tensor_copy(
                                    apv(st, ci, [[128, MH], [4, HD]]),
                                    apv(src, 0, [[32, MH], [1, HD]]))
                            nc.sync.dma_start(
                                tables[TB_L[l] + it * 128:TB_L[l] + (it + 1) * 128, :],
                                st[:])

            if DEBUG:
                nc.sync.dma_start(dbg_tbl[:], tables[TB_L[0] + 256:TB_L[0] + 384, :])
            # ================= S3: per-chunk main loop =================
            tbl_rows = tables[:].rearrange("t (a k) -> (t a) k", a=8, k=HD * 4)
            with (
                tc.tile_pool(name="mpsum", bufs=2, space="PSUM") as mp,
                tc.tile_pool(name="math", bufs=3) as mt,
                tc.tile_pool(name="gath", bufs=3) as gp,
                tc.tile_pool(name="red", bufs=3) as rdp,
                tc.tile_pool(name="opsum", bufs=2, space="PSUM") as op,
            ):
                for ch in range(NCHUNK):
                    qs = ch * 128

                    def mm3(ps, wt, wb, n):
                        nc.tensor.matmul(ps, qT_sb[:, qs:qs + 128],
                                         wt[:, 0:n], start=True, stop=False)
                        nc.tensor.matmul(ps, qT_sb[:, QPC + qs:QPC + qs + 128],
                                         wt[:, n:2 * n], start=False, stop=False)
                        nc.tensor.matmul(ps, ones_sb[:], wb[:],
                                         start=False, stop=True)
                        return ps

                    p_off = mm3(mp.tile([128, 256], F32, tag="poff", name="poff")[:],
                                wo_sb, wo_b, 256)
                    p_att = mm3(mp.tile([128, 128], F32, tag="patt", name="patt")[:],
                                wa_sb, wa_b, 128)

                    # softmax over 16 lp per head (logits are small: no max-sub)
                    aexp = mt.tile([128, 128], F32, tag="aexp")
                    nc.scalar.activation(aexp[:], p_att, AF.Exp)
                    asum = mt.tile([128, 8], F32, tag="asum")
                    nc.vector.tensor_reduce(
                        asum[:], apv(aexp, 0, [[16, 8], [1, 16]]), AX.X, ALU.add)
                    arec = mt.tile([128, 8], F32, tag="arec")
                    nc.vector.reciprocal(arec[:], asum[:])
                    aw = mt.tile([128, 128], F32, tag="aw")
                    nc.vector.tensor_tensor(
                        apv(aw, 0, [[16, 8], [1, 16]]),
                        apv(aexp, 0, [[16, 8], [1, 16]]),
                        apv(arec, 0, [[1, 8], [0, 16]]), ALU.mult)

                    # ---- locations: xy = p_off + refsW (bcast over h) ----
                    xy = mt.tile([128, 256], F32, tag="xy")
                    nc.vector.tensor_tensor(
                        apv(xy, 0, [[32, 8], [1, 32]]),
                        apv(p_off, 0, [[32, 8], [1, 32]]),
                        apv(refsW_sb[:, ch * 32:(ch + 1) * 32], 0, [[0, 8], [1, 32]]),
                        ALU.add)
                    # floor(x), robust to cast rounding mode (trunc or nearest):
                    # x0c = cast(x); l = x - x0c; if l < 0: x0c -= 1, l += 1
                    xyi = mt.tile([128, 256], I32, tag="xyi")
                    nc.vector.tensor_scalar(xyi[:], xy[:], 0.0, None, ALU.add)
                    xy0 = mt.tile([128, 256], F32, tag="xy0")
                    nc.vector.tensor_scalar(xy0[:], xyi[:], 0.0, None, ALU.add)
                    lxy = mt.tile([128, 256], F32, tag="lxy")
                    nc.vector.tensor_tensor(lxy[:], xy[:], xy0[:], ALU.subtract)
                    neg = mt.tile([128, 256], F32, tag="neg")
                    nc.vector.tensor_scalar(neg[:], lxy[:], 0.0, None, ALU.is_lt)
                    nc.vector.tensor_tensor(xy0[:], xy0[:], neg[:], ALU.subtract)
                    nc.vector.tensor_tensor(lxy[:], lxy[:], neg[:], ALU.add)
                    oml = mt.tile([128, 256], F32, tag="oml")
                    nc.scalar.activation(oml[:], lxy[:], AF.Copy, bias=1.0, scale=-1.0)

                    # validity masks: v0 for x0/y0, v1 for x0+1/y0+1
                    hlpxy = lambda t: apv(t, 0, [[32, 8], [8, 4], [1, 8]])
                    cl = lambda n: apv(consts[n], 0, [[0, 8], [1, 4], [0, 8]])
                    v0 = mt.tile([128, 256], F32, tag="v0")
                    nc.vector.tensor_tensor(hlpxy(v0), hlpxy(xy0), cl("cWm1"),
                                            ALU.is_le)
                    nc.vector.scalar_tensor_tensor(v0[:], xy0[:], -0.5, v0[:],
                                                   ALU.is_ge, ALU.mult)
                    v1 = mt.tile([128, 256], F32, tag="v1")
                    nc.vector.tensor_tensor(hlpxy(v1), hlpxy(xy0), cl("cWm2"),
                                            ALU.is_le)
                    nc.vector.scalar_tensor_tensor(v1[:], xy0[:], -1.5, v1[:],
                                                   ALU.is_ge, ALU.mult)
                    wA = mt.tile([128, 256], F32, tag="wA")
                    nc.vector.tensor_tensor(wA[:], oml[:], v0[:], ALU.mult)
                    wB = mt.tile([128, 256], F32, tag="wB")
                    nc.vector.tensor_tensor(wB[:], lxy[:], v1[:], ALU.mult)

                    # fold attention weight into y-weights (y at odd offsets)
                    wy0a = mt.tile([128, 128], F32, tag="wy0a")
                    nc.vector.tensor_tensor(wy0a[:], apv(wA, 1, [[2, 128]]),
                                            aw[:], ALU.mult)
                    wy1a = mt.tile([128, 128], F32, tag="wy1a")
                    nc.vector.tensor_tensor(wy1a[:], apv(wB, 1, [[2, 128]]),
                                            aw[:], ALU.mult)

                    # corner weights w4 (slot, c) bf16: c = cy*2 + cx
                    w4 = mt.tile([128, 512], BF16, tag="w4")
                    for c, (wy, wx) in enumerate(
                            [(wy0a, wA), (wy0a, wB), (wy1a, wA), (wy1a, wB)]):
                        nc.vector.tensor_tensor(
                            apv(w4, c, [[4, 128]]), wy[:],
                            apv(wx, 0, [[2, 128]]), ALU.mult)

                    # table row index: t = clamp(y0*W + x0); idx = t*8 + C8h
                    hlp = lambda t, o=0: apv(t, o, [[32, 8], [8, 4], [2, 4]])
                    cv4 = lambda n: apv(consts[n], 0, [[0, 8], [1, 4], [0, 4]])
                    t1 = mt.tile([128, 128], F32, tag="t1")
                    t1v = apv(t1, 0, [[16, 8], [4, 4], [1, 4]])
                    nc.vector.tensor_tensor(t1v, hlp(xy0, 1), cv4("cW"), ALU.mult)
                    nc.vector.tensor_tensor(t1v, t1v, hlp(xy0, 0), ALU.add)
                    nc.vector.tensor_tensor(t1v, t1v, cv4("cLo"), ALU.max)
                    nc.vector.tensor_tensor(t1v, t1v, cv4("cHi"), ALU.min)
                    nc.vector.tensor_scalar(t1[:], t1[:], 8.0, None, ALU.mult)
                    idx = mt.tile([128, 128], I32, tag="idx")
                    nc.vector.tensor_tensor(
                        apv(idx, 0, [[16, 8], [4, 4], [1, 4]]), t1v,
                        apv(consts["cC8h"], 0, [[4, 8], [1, 4], [0, 4]]), ALU.add)

                    if DEBUG and ch == 0:
                        nc.sync.dma_start(dbg_idx[:], idx[:])
                        nc.sync.dma_start(dbg_w4[:], w4[:])
                    # ---- gather + weighted reduce, two 4-head halves ----
                    smpl = mt.tile([128, 256], F32, tag="smpl")
                    for half in range(2):
                        g = gp.tile([128, 64 * 128], BF16, tag="g")
                        for s in range(64):
                            nc.gpsimd.indirect_dma_start(
                                g[:, s * 128:(s + 1) * 128],
                                None,
                                tbl_rows,
                                IndirectOffsetOnAxis(
                                    ap=idx[:, half * 64 + s:half * 64 + s + 1],
                                    axis=0),
                            )
                        if DEBUG and ch == 0 and half == 0:
                            nc.sync.dma_start(dbg_g[:], g[:])
                        gv = apv(g, 0, [[128, 64], [4, HD], [1, 4]])
                        nc.vector.tensor_tensor(
                            gv, gv,
                            apv(w4[:, half * 256:(half + 1) * 256], 0,
                                [[4, 64], [0, HD], [1, 4]]),
                            ALU.mult)
                        r1 = rdp.tile([128, 64 * 32], BF16, tag="r1")
                        with nc.allow_low_precision("bf16 corner-sum"):
                            nc.vector.tensor_reduce(
                                apv(r1, 0, [[32, 64], [1, 32]]), gv, AX.X, ALU.add)
                        nc.vector.tensor_reduce(
                            apv(smpl[:, half * 128:(half + 1) * 128], 0,
                                [[32, 4], [1, 32]]),
                            apv(r1, 0, [[512, 4], [1, 32], [32, 16]]),
                            AX.X, ALU.add)

                    if DEBUG and ch == 0:
                        nc.sync.dma_start(dbg_smpl[:], smpl[:])
                    # ---- S4 out projection ----
                    smplT = mt.tile([128, 256], F32, tag="smplT")
                    for half in range(2):
                        pt = op.tile([128, 128], F32, tag="pt")
                        nc.tensor.transpose(
                            pt[:], smpl[:, half * 128:(half + 1) * 128], id_sb[:])
                        nc.vector.tensor_copy(
                            smplT[:, half * 128:(half + 1) * 128], pt[:])
                    po = op.tile([128, 256], F32, tag="po")
                    nc.tensor.matmul(po[:], smplT[:, 0:128], wq_sb[:, 0:256],
                                     start=True, stop=False)
                    nc.tensor.matmul(po[:], smplT[:, 128:256], wq_sb[:, 256:512],
                                     start=False, stop=False)
                    nc.tensor.matmul(po[:], ones_sb[:], wq_b[:],
                                     start=False, stop=True)
                    osb = mt.tile([128, 256], F32, tag="osb")
                    nc.scalar.activation(osb[:], po[:], AF.Copy)
                    nc.sync.dma_start(out_d[qs:qs + 128, :], osb[:])

    nc.compile()
    return nc


def _host_consts():
    Wl = np.array([w for (h, w) in SHAPES], np.float32)  # W == H per level
    rep = lambda v: np.ascontiguousarray(
        np.broadcast_to(np.asarray(v, np.float32)[None, :], (128, len(v))))
    c = {}
    c["cW"] = rep(Wl)
    c["cWm1"] = rep(Wl - 1.0)
    c["cWm2"] = rep(Wl - 2.0)
    c["cLo"] = rep(-np.array(OFF_L, np.float32))
    c["cHi"] = rep(np.array([h * w for (h, w) in SHAPES], np.float32) - 1.0)
    C8h = np.zeros((MH, NL), np.float32)
    for h in range(MH):
        for l in range(NL):
            C8h[h, l] = (TB_L[l] + OFF_L[l]) * 8 + h
    c["cC8h"] = rep(C8h.reshape(-1))
    pm = np.zeros((len(SHIFTS), 2, 128, 128), np.float32)
    for i, s in enumerate(SHIFTS):
        for m in range(128 - s):
            pm[i, 0, m + s, m] = 1.0
        for m in range(128 - s, 128):
            pm[i, 1, m - (128 - s), m] = 1.0
    c["pmats"] = pm.astype(ml_dtypes.bfloat16).reshape(len(SHIFTS) * 2 * 128, 128)
    c["ident"] = np.eye(128, dtype=np.float32)
    return c


def _prep_core_inputs(c, ins, consts):
    b, qh = c // 2, c % 2
    q0 = qh * QPC
    qT = np.ascontiguousarray(ins["query"][b, q0:q0 + QPC].T)
    xT = np.ascontiguousarray(ins["input_flatten"][b].T)
    refs = ins["reference_points"][b, q0:q0 + QPC]  # (QPC, NL, 2)
    WH = np.array([[w, h] for (h, w) in SHAPES], np.float32)  # (NL, 2) = (W, H)
    rw = (refs * WH[None]).astype(np.float32)  # -0.5 is folded into b_off
    rw = np.repeat(rw[:, :, None, :], NP, axis=2).reshape(QPC, 32)
    refsW = np.ascontiguousarray(
        rw.reshape(NCHUNK, 128, 32).transpose(1, 0, 2).reshape(128, NCHUNK * 32))
    aug = lambda W, bb: np.vstack([W, np.asarray(bb, np.float32)[None, :]]).astype(
        np.float32)
    m = {
        "qT": qT, "xT": xT, "refsW": refsW,
        "Woff": aug(ins["W_off"], np.asarray(ins["b_off"], np.float32) - 0.5),
        "Wattn": aug(ins["W_attn"], ins["b_attn"]),
        "Wval": np.asarray(ins["W_val"], np.float32),
        "bval": np.ascontiguousarray(
            np.broadcast_to(np.asarray(ins["b_val"], np.float32)[None, :],
                            (128, 256))),
        "Wout": aug(ins["W_out"], ins["b_out"]),
        "pmats": consts["pmats"], "ident": consts["ident"],
    }
    for k in ("cW", "cWm1", "cWm2", "cLo", "cHi", "cC8h"):
        m[k] = consts[k]
    return m


_NC_CACHE = {}


def kernel(**inputs):
    ins = {k: np.asarray(v) for k, v in inputs.items()}
    if "nc" not in _NC_CACHE:
        _NC_CACHE["nc"] = build_nc()
    nc = _NC_CACHE["nc"]
    consts = _host_consts()
    in_maps = [_prep_core_inputs(c, ins, consts) for c in range(N_CORES)]
    res = run_bass_kernel_spmd(nc, in_maps, core_ids=list(range(N_CORES)))
    out = np.zeros((N, LQ, DM), np.float32)
    for c in range(N_CORES):
        b, qh = c // 2, c % 2
        out[b, qh * QPC:(qh + 1) * QPC] = res.results[c]["out"]
    return out



# revision 20
# speedup vs baseline: 1.2027x; 1.2027x over previous
"""Deformable-DETR multi-scale attention kernel for 8 Trainium2 NeuronCores.

Sharding: core c handles batch b=c//2, query half (c%2)*4096..+4096.
Per-core pipeline:
  S1 value projection (PE) -> value_sb bf16 [tok%128, (tok//128)*256 + dim]
  S2 build 4-corner "patch" tables in HBM: row (t,h) = [hd, corner] bf16
     via PE partition-rotations + strided DVE copies + DMA
  S3 per 128-query chunk: off/attn projections, softmax, bilinear weights
     + table indices, indirect-DMA gather, weighted reduce (DVE)
  S4 out projection (PE) -> out rows
Host: transpose/shard inputs, assemble output.
"""

import numpy as np
import ml_dtypes

import concourse.bass as bass
import concourse.mybir as mybir
import concourse.tile as tile
from concourse import bacc
from concourse.bass import IndirectOffsetOnAxis
from concourse.bass_utils import run_bass_kernel_spmd

F32 = mybir.dt.float32
BF16 = mybir.dt.bfloat16
I32 = mybir.dt.int32
I16 = mybir.dt.int16
AF = mybir.ActivationFunctionType
ALU = mybir.AluOpType
AX = mybir.AxisListType

# Problem constants
N, LQ, DM, MH, NL, NP = 4, 8192, 256, 8, 4, 4
HD = DM // MH  # 32
SHAPES = [(128, 128), (64, 64), (32, 32), (16, 16)]
S = sum(h * w for h, w in SHAPES)  # 21760
LS = [0, 16384, 20480, 21504]  # level starts in value
N_CORES = 8
QPC = LQ // 2  # queries per core = 4096
NCHUNK = QPC // 128  # 32

# Table geometry: per level, entry t <-> pixel p = t - OFF_L[l]
# patch(t) = value rows [p, p+1, p+W, p+W+1], stored as a row of
# (h, hd, corner)-ordered bf16: 1024 els = 2KB per row, 128 els per head.
OFF_L = [256, 128, 128, 128]  # multiples of 128, >= W+1
TROWS_L = [16640, 4224, 1152, 384]  # HW + OFF_L, multiples of 128
TB_L = [0, 16640, 20864, 22016]
TOT_ROWS = 22400
ROW_ELS = MH * HD * 4  # 1024
# value_sb covers tok in [-256, S+256): 174 tiles of 128
VPAD = 256
VTILES = (S + 2 * VPAD) // 128  # 174

SHIFTS = [1, 16, 17, 32, 33, 64, 65]
DEBUG = False


def apv(src, extra_offset, dims):
    """Strided free-dim view: keep partition dim, replace free dims."""
    a = src if isinstance(src, bass.AP) else src[:]
    return bass.AP(a.tensor, a.offset + extra_offset, [a.ap[0]] + list(dims))


def build_nc():
    nc = bacc.Bacc("TRN2", target_bir_lowering=False, debug=False,
                   num_devices=N_CORES)

    # ---- dram I/O ----
    qT = nc.dram_tensor("qT", [DM, QPC], F32, kind="ExternalInput")
    xT = nc.dram_tensor("xT", [DM, S], F32, kind="ExternalInput")
    refsW = nc.dram_tensor("refsW", [128, NCHUNK * 32], F32, kind="ExternalInput")
    Woff = nc.dram_tensor("Woff", [DM + 1, 256], F32, kind="ExternalInput")
    Wattn = nc.dram_tensor("Wattn", [DM + 1, 128], F32, kind="ExternalInput")
    Wval = nc.dram_tensor("Wval", [DM, 256], F32, kind="ExternalInput")
    bval = nc.dram_tensor("bval", [128, 256], F32, kind="ExternalInput")
    Wout = nc.dram_tensor("Wout", [DM + 1, 256], F32, kind="ExternalInput")
    cdefs = {"cW": 4, "cWm1": 4, "cWm2": 4, "cLo": 4, "cHi": 4, "cC8h": 32}
    cdram = {k: nc.dram_tensor(k, [128, w], F32, kind="ExternalInput")
             for k, w in cdefs.items()}
    pmats = nc.dram_tensor("pmats", [len(SHIFTS) * 2 * 128, 128], BF16,
                           kind="ExternalInput")
    ident = nc.dram_tensor("ident", [128, 128], F32, kind="ExternalInput")
    out_d = nc.dram_tensor("out", [QPC, 256], F32, kind="ExternalOutput")
    if DEBUG:
        dbg_idx = nc.dram_tensor("dbg_idx", [128, 128], I32, kind="ExternalOutput")
        dbg_g = nc.dram_tensor("dbg_g", [128, 8192], BF16, kind="ExternalOutput")
        dbg_w4 = nc.dram_tensor("dbg_w4", [128, 512], BF16, kind="ExternalOutput")
        dbg_smpl = nc.dram_tensor("dbg_smpl", [128, 256], F32, kind="ExternalOutput")
        dbg_val = nc.dram_tensor("dbg_val", [128, 1024], BF16, kind="ExternalOutput")
        dbg_tbl = nc.dram_tensor("dbg_tbl", [128, ROW_ELS], BF16, kind="ExternalOutput")

    with tile.TileContext(nc) as tc:
        with (
            tc.tile_pool(name="persist", bufs=1) as pp,
            tc.tile_pool(name="dram", bufs=1, space="DRAM") as dp,
        ):
            tables = dp.tile([TOT_ROWS, ROW_ELS], BF16, tag="tables")

            # persistent SBUF tiles
            qT_sb = pp.tile([128, 2 * QPC], F32, tag="qT")
            nc.sync.dma_start(qT_sb[:, 0:QPC], qT[0:128, :])
            nc.sync.dma_start(qT_sb[:, QPC:], qT[128:256, :])
            refsW_sb = pp.tile([128, NCHUNK * 32], F32, tag="refsW")
            nc.sync.dma_start(refsW_sb[:], refsW[:])
            wv_sb = pp.tile([128, 2 * 256], F32, tag="wv")
            nc.sync.dma_start(wv_sb[:, 0:256], Wval[0:128, :])
            nc.sync.dma_start(wv_sb[:, 256:], Wval[128:256, :])
            bval_sb = pp.tile([128, 256], F32, tag="bval")
            nc.sync.dma_start(bval_sb[:], bval[:])
            wo_sb = pp.tile([128, 2 * 256], F32, tag="wo")
            nc.sync.dma_start(wo_sb[:, 0:256], Woff[0:128, :])
            nc.sync.dma_start(wo_sb[:, 256:512], Woff[128:256, :])
            wo_b = pp.tile([1, 256], F32, tag="wo_b")
            nc.sync.dma_start(wo_b[:], Woff[256:257, :])
            wa_sb = pp.tile([128, 2 * 128], F32, tag="wa")
            nc.sync.dma_start(wa_sb[:, 0:128], Wattn[0:128, :])
            nc.sync.dma_start(wa_sb[:, 128:256], Wattn[128:256, :])
            wa_b = pp.tile([1, 128], F32, tag="wa_b")
            nc.sync.dma_start(wa_b[:], Wattn[256:257, :])
            wq_sb = pp.tile([128, 2 * 256], F32, tag="wq")
            nc.sync.dma_start(wq_sb[:, 0:256], Wout[0:128, :])
            nc.sync.dma_start(wq_sb[:, 256:512], Wout[128:256, :])
            wq_b = pp.tile([1, 256], F32, tag="wq_b")
            nc.sync.dma_start(wq_b[:], Wout[256:257, :])
            consts = {}
            for name, w in cdefs.items():
                consts[name] = pp.tile([128, w], F32, tag=name, name=name + "_sb")
                nc.sync.dma_start(consts[name][:], cdram[name][:])
            pm_sb = pp.tile([128, len(SHIFTS) * 2 * 128], BF16, tag="pm")
            for i in range(len(SHIFTS) * 2):
                nc.sync.dma_start(pm_sb[:, i * 128:(i + 1) * 128],
                                  pmats[i * 128:(i + 1) * 128, :])
            id_sb = pp.tile([128, 128], F32, tag="ident")
            nc.sync.dma_start(id_sb[:], ident[:])
            ones_sb = pp.tile([1, 128], F32, tag="ones")
            nc.vector.memset(ones_sb[:], 1.0)

            def pmat(s, which):  # which: 0=P (main), 1=Q (boundary)
                i = SHIFTS.index(s) * 2 + which
                return pm_sb[:, i * 128:(i + 1) * 128]

            with tc.tile_pool(name="vsb", bufs=1) as vp:
                value_sb = vp.tile([128, VTILES * 256], BF16, tag="value")

                def vtile(k):  # value_sb tile k (tok = (k-2)*128 + part)
                    assert 0 <= k < VTILES, k
                    return value_sb[:, k * 256:(k + 1) * 256]

                # ================= S1: value projection =================
                with (
                    tc.tile_pool(name="xslab", bufs=2) as xp,
                    tc.tile_pool(name="vpsum", bufs=4, space="PSUM") as vpp,
                ):
                    nc.vector.memset(value_sb[:, 0:512], 0.0)
                    nc.vector.memset(value_sb[:, (VTILES - 2) * 256:], 0.0)
                    SLAB = 2048
                    for s0 in range(0, S, SLAB):
                        sl = min(SLAB, S - s0)
                        xt = xp.tile([128, 2 * SLAB], F32, tag="xslab")
                        nc.sync.dma_start(xt[:, 0:sl], xT[0:128, s0:s0 + sl])
                        nc.sync.dma_start(xt[:, SLAB:SLAB + sl],
                                          xT[128:256, s0:s0 + sl])
                        for t0 in range(0, sl, 128):
                            pv = vpp.tile([128, 256], F32, tag="vpsum")
                            nc.tensor.matmul(pv[:], xt[:, t0:t0 + 128],
                                             wv_sb[:, 0:256], start=True, stop=False)
                            nc.tensor.matmul(pv[:], xt[:, SLAB + t0:SLAB + t0 + 128],
                                             wv_sb[:, 256:512], start=False, stop=True)
                            vt = (s0 + t0) // 128 + 2
                            nc.vector.tensor_tensor(
                                value_sb[:, vt * 256:(vt + 1) * 256], pv[:],
                                bval_sb[:], ALU.add)

                if DEBUG:
                    nc.sync.dma_start(dbg_val[:, 0:512], value_sb[:, 2 * 256:4 * 256])
                    nc.sync.dma_start(dbg_val[:, 512:1024],
                                      value_sb[:, 130 * 256:132 * 256])
                # ================= S2: patch tables =================
                with (
                    tc.tile_pool(name="stage", bufs=3) as sp,
                    tc.tile_pool(name="rpsum", bufs=6, space="PSUM") as rp,
                ):
                    def rot(s, k):
                        """rows (k*128 + part + s) of padded value."""
                        pr = rp.tile([128, 256], F32, tag="rot")
                        nc.tensor.matmul(pr[:], pmat(s, 0), vtile(k),
                                         start=True, stop=False)
                        nc.tensor.matmul(pr[:], pmat(s, 1), vtile(k + 1),
                                         start=False, stop=True)
                        return pr[:]

                    for l, (H, W) in enumerate(SHAPES):
                        ntile = TROWS_L[l] // 128
                        k0 = (LS[l] - OFF_L[l] + VPAD) // 128
                        for it in range(ntile):
                            st = sp.tile([128, ROW_ELS], BF16, tag="stage")
                            if W == 128:
                                srcs = [vtile(k0 + it), rot(1, k0 + it),
                                        vtile(k0 + it + 1), rot(1, k0 + it + 1)]
                            else:
                                srcs = [vtile(k0 + it), rot(1, k0 + it),
                                        rot(W, k0 + it), rot(W + 1, k0 + it)]
                            for ci, src in enumerate(srcs):
                                nc.vector.tensor_copy(
                                    apv(st, ci, [[128, MH], [4, HD]]),
                                    apv(src, 0, [[32, MH], [1, HD]]))
                            nc.sync.dma_start(
                                tables[TB_L[l] + it * 128:TB_L[l] + (it + 1) * 128, :],
                                st[:])

            if DEBUG:
                nc.sync.dma_start(dbg_tbl[:], tables[TB_L[0] + 256:TB_L[0] + 384, :])
            # ================= S3: per-chunk main loop =================
            tbl_rows = tables[:].rearrange("t (a k) -> (t a) k", a=8, k=HD * 4)
            with (
                tc.tile_pool(name="mpsum", bufs=2, space="PSUM") as mp,
                tc.tile_pool(name="math", bufs=3) as mt,
                tc.tile_pool(name="gath", bufs=3) as gp,
                tc.tile_pool(name="red", bufs=3) as rdp,
                tc.tile_pool(name="opsum", bufs=2, space="PSUM") as op,
            ):
                for ch in range(NCHUNK):
                    qs = ch * 128

                    def mm3(ps, wt, wb, n):
                        nc.tensor.matmul(ps, qT_sb[:, qs:qs + 128],
                                         wt[:, 0:n], start=True, stop=False)
                        nc.tensor.matmul(ps, qT_sb[:, QPC + qs:QPC + qs + 128],
                                         wt[:, n:2 * n], start=False, stop=False)
                        nc.tensor.matmul(ps, ones_sb[:], wb[:],
                                         start=False, stop=True)
                        return ps

                    p_off = mm3(mp.tile([128, 256], F32, tag="poff", name="poff")[:],
                                wo_sb, wo_b, 256)
                    p_att = mm3(mp.tile([128, 128], F32, tag="patt", name="patt")[:],
                                wa_sb, wa_b, 128)

                    # softmax over 16 lp per head (logits are small: no max-sub)
                    aexp = mt.tile([128, 128], F32, tag="aexp")
                    nc.scalar.activation(aexp[:], p_att, AF.Exp)
                    asum = mt.tile([128, 8], F32, tag="asum")
                    nc.vector.tensor_reduce(
                        asum[:], apv(aexp, 0, [[16, 8], [1, 16]]), AX.X, ALU.add)
                    arec = mt.tile([128, 8], F32, tag="arec")
                    nc.vector.reciprocal(arec[:], asum[:])
                    aw = mt.tile([128, 128], F32, tag="aw")
                    nc.vector.tensor_tensor(
                        apv(aw, 0, [[16, 8], [1, 16]]),
                        apv(aexp, 0, [[16, 8], [1, 16]]),
                        apv(arec, 0, [[1, 8], [0, 16]]), ALU.mult)

                    # ---- locations: xy = p_off + refsW (bcast over h) ----
                    xy = mt.tile([128, 256], F32, tag="xy")
                    nc.vector.tensor_tensor(
                        apv(xy, 0, [[32, 8], [1, 32]]),
                        apv(p_off, 0, [[32, 8], [1, 32]]),
                        apv(refsW_sb[:, ch * 32:(ch + 1) * 32], 0, [[0, 8], [1, 32]]),
                        ALU.add)
                    # floor(x), robust to cast rounding mode (trunc or nearest):
                    # x0c = cast(x); l = x - x0c; if l < 0: x0c -= 1, l += 1
                    xyi = mt.tile([128, 256], I32, tag="xyi")
                    nc.vector.tensor_scalar(xyi[:], xy[:], 0.0, None, ALU.add)
                    xy0 = mt.tile([128, 256], F32, tag="xy0")
                    nc.vector.tensor_scalar(xy0[:], xyi[:], 0.0, None, ALU.add)
                    lxy = mt.tile([128, 256], F32, tag="lxy")
                    nc.vector.tensor_tensor(lxy[:], xy[:], xy0[:], ALU.subtract)
                    neg = mt.tile([128, 256], F32, tag="neg")
                    nc.vector.tensor_scalar(neg[:], lxy[:], 0.0, None, ALU.is_lt)
                    nc.vector.tensor_tensor(xy0[:], xy0[:], neg[:], ALU.subtract)
                    nc.vector.tensor_tensor(lxy[:], lxy[:], neg[:], ALU.add)
                    oml = mt.tile([128, 256], F32, tag="oml")
                    nc.scalar.activation(oml[:], lxy[:], AF.Copy, bias=1.0, scale=-1.0)

                    # validity masks: v0 for x0/y0, v1 for x0+1/y0+1
                    hlpxy = lambda t: apv(t, 0, [[32, 8], [8, 4], [1, 8]])
                    cl = lambda n: apv(consts[n], 0, [[0, 8], [1, 4], [0, 8]])
                    v0 = mt.tile([128, 256], F32, tag="v0")
                    nc.vector.tensor_tensor(hlpxy(v0), hlpxy(xy0), cl("cWm1"),
                                            ALU.is_le)
                    nc.vector.scalar_tensor_tensor(v0[:], xy0[:], -0.5, v0[:],
                                                   ALU.is_ge, ALU.mult)
                    v1 = mt.tile([128, 256], F32, tag="v1")
                    nc.vector.tensor_tensor(hlpxy(v1), hlpxy(xy0), cl("cWm2"),
                                            ALU.is_le)
                    nc.vector.scalar_tensor_tensor(v1[:], xy0[:], -1.5, v1[:],
                                                   ALU.is_ge, ALU.mult)
                    wA = mt.tile([128, 256], F32, tag="wA")
                    nc.vector.tensor_tensor(wA[:], oml[:], v0[:], ALU.mult)
                    wB = mt.tile([128, 256], F32, tag="wB")
                    nc.vector.tensor_tensor(wB[:], lxy[:], v1[:], ALU.mult)

                    # fold attention weight into y-weights (y at odd offsets)
                    wy0a = mt.tile([128, 128], F32, tag="wy0a")
                    nc.vector.tensor_tensor(wy0a[:], apv(wA, 1, [[2, 128]]),
                                            aw[:], ALU.mult)
                    wy1a = mt.tile([128, 128], F32, tag="wy1a")
                    nc.vector.tensor_tensor(wy1a[:], apv(wB, 1, [[2, 128]]),
                                            aw[:], ALU.mult)

                    # corner weights w4 (slot, c) bf16: c = cy*2 + cx
                    w4 = mt.tile([128, 512], BF16, tag="w4")
                    for c, (wy, wx) in enumerate(
                            [(wy0a, wA), (wy0a, wB), (wy1a, wA), (wy1a, wB)]):
                        nc.vector.tensor_tensor(
                            apv(w4, c, [[4, 128]]), wy[:],
                            apv(wx, 0, [[2, 128]]), ALU.mult)

                    # table row index: t = clamp(y0*W + x0); idx = t*8 + C8h
                    hlp = lambda t, o=0: apv(t, o, [[32, 8], [8, 4], [2, 4]])
                    cv4 = lambda n: apv(consts[n], 0, [[0, 8], [1, 4], [0, 4]])
                    t1 = mt.tile([128, 128], F32, tag="t1")
                    t1v = apv(t1, 0, [[16, 8], [4, 4], [1, 4]])
                    nc.vector.tensor_tensor(t1v, hlp(xy0, 1), cv4("cW"), ALU.mult)
                    nc.vector.tensor_tensor(t1v, t1v, hlp(xy0, 0), ALU.add)
                    nc.vector.tensor_tensor(t1v, t1v, cv4("cLo"), ALU.max)
                    nc.vector.tensor_tensor(t1v, t1v, cv4("cHi"), ALU.min)
                    nc.vector.tensor_scalar(t1[:], t1[:], 8.0, None, ALU.mult)
                    idx = mt.tile([128, 128], I32, tag="idx")
                    nc.vector.tensor_tensor(
                        apv(idx, 0, [[16, 8], [4, 4], [1, 4]]), t1v,
                        apv(consts["cC8h"], 0, [[4, 8], [1, 4], [0, 4]]), ALU.add)

                    if DEBUG and ch == 0:
                        nc.sync.dma_start(dbg_w4[:], w4[:])
                    # ---- gather + weighted reduce, two 4-head halves ----
                    smpl = mt.tile([128, 256], F32, tag="smpl")
                    for half in range(2):
                        g = gp.tile([128, 64 * 128], BF16, tag="g")
                        for s in range(64):
                            nc.gpsimd.indirect_dma_start(
                                g[:, s * 128:(s + 1) * 128],
                                None,
                                tbl_rows,
                                IndirectOffsetOnAxis(
                                    ap=idx[:, half * 64 + s:half * 64 + s + 1],
                                    axis=0),
                            )
                        if DEBUG and ch == 0 and half == 0:
                            nc.sync.dma_start(dbg_g[:], g[:])
                        gv = apv(g, 0, [[128, 64], [4, HD], [1, 4]])
                        nc.vector.tensor_tensor(
                            gv, gv,
                            apv(w4[:, half * 256:(half + 1) * 256], 0,
                                [[4, 64], [0, HD], [1, 4]]),
                            ALU.mult)
                        r1 = rdp.tile([128, 64 * 32], BF16, tag="r1")
                        with nc.allow_low_precision("bf16 corner-sum"):
                            nc.vector.tensor_reduce(
                                apv(r1, 0, [[32, 64], [1, 32]]), gv, AX.X, ALU.add)
                        nc.vector.tensor_reduce(
                            apv(smpl[:, half * 128:(half + 1) * 128], 0,
                                [[32, 4], [1, 32]]),
                            apv(r1, 0, [[512, 4], [1, 32], [32, 16]]),
                            AX.X, ALU.add)

                    if DEBUG and ch == 0:
                        nc.sync.dma_start(dbg_smpl[:], smpl[:])
                    # ---- S4 out projection ----
                    smplT = mt.tile([128, 256], F32, tag="smplT")
                    for half in range(2):
                        pt = op.tile([128, 128], F32, tag="pt")
                        nc.tensor.transpose(
                            pt[:], smpl[:, half * 128:(half + 1) * 128], id_sb[:])
                        nc.vector.tensor_copy(
                            smplT[:, half * 128:(half + 1) * 128], pt[:])
                    po = op.tile([128, 256], F32, tag="po")
                    nc.tensor.matmul(po[:], smplT[:, 0:128], wq_sb[:, 0:256],
                                     start=True, stop=False)
                    nc.tensor.matmul(po[:], smplT[:, 128:256], wq_sb[:, 256:512],
                                     start=False, stop=False)
                    nc.tensor.matmul(po[:], ones_sb[:], wq_b[:],
                                     start=False, stop=True)
                    osb = mt.tile([128, 256], F32, tag="osb")
                    nc.scalar.activation(osb[:], po[:], AF.Copy)
                    nc.sync.dma_start(out_d[qs:qs + 128, :], osb[:])

    nc.compile()
    return nc


def _host_consts():
    Wl = np.array([w for (h, w) in SHAPES], np.float32)  # W == H per level
    rep = lambda v: np.ascontiguousarray(
        np.broadcast_to(np.asarray(v, np.float32)[None, :], (128, len(v))))
    c = {}
    c["cW"] = rep(Wl)
    c["cWm1"] = rep(Wl - 1.0)
    c["cWm2"] = rep(Wl - 2.0)
    c["cLo"] = rep(-np.array(OFF_L, np.float32))
    c["cHi"] = rep(np.array([h * w for (h, w) in SHAPES], np.float32) - 1.0)
    C8h = np.zeros((MH, NL), np.float32)
    for h in range(MH):
        for l in range(NL):
            C8h[h, l] = (TB_L[l] + OFF_L[l]) * 8 + h
    c["cC8h"] = rep(C8h.reshape(-1))
    pm = np.zeros((len(SHIFTS), 2, 128, 128), np.float32)
    for i, s in enumerate(SHIFTS):
        for m in range(128 - s):
            pm[i, 0, m + s, m] = 1.0
        for m in range(128 - s, 128):
            pm[i, 1, m - (128 - s), m] = 1.0
    c["pmats"] = pm.astype(ml_dtypes.bfloat16).reshape(len(SHIFTS) * 2 * 128, 128)
    c["ident"] = np.eye(128, dtype=np.float32)
    return c


def _prep_core_inputs(c, ins, consts):
    b, qh = c // 2, c % 2
    q0 = qh * QPC
    qT = np.ascontiguousarray(ins["query"][b, q0:q0 + QPC].T)
    xT = np.ascontiguousarray(ins["input_flatten"][b].T)
    refs = ins["reference_points"][b, q0:q0 + QPC]  # (QPC, NL, 2)
    WH = np.array([[w, h] for (h, w) in SHAPES], np.float32)  # (NL, 2) = (W, H)
    rw = (refs * WH[None]).astype(np.float32)  # -0.5 is folded into b_off
    rw = np.repeat(rw[:, :, None, :], NP, axis=2).reshape(QPC, 32)
    refsW = np.ascontiguousarray(
        rw.reshape(NCHUNK, 128, 32).transpose(1, 0, 2).reshape(128, NCHUNK * 32))
    aug = lambda W, bb: np.vstack([W, np.asarray(bb, np.float32)[None, :]]).astype(
        np.float32)
    m = {
        "qT": qT, "xT": xT, "refsW": refsW,
        "Woff": aug(ins["W_off"], np.asarray(ins["b_off"], np.float32) - 0.5),
        "Wattn": aug(ins["W_attn"], ins["b_attn"]),
        "Wval": np.asarray(ins["W_val"], np.float32),
        "bval": np.ascontiguousarray(
            np.broadcast_to(np.asarray(ins["b_val"], np.float32)[None, :],
                            (128, 256))),
        "Wout": aug(ins["W_out"], ins["b_out"]),
        "pmats": consts["pmats"], "ident": consts["ident"],
    }
    for k in ("cW", "cWm1", "cWm2", "cLo", "cHi", "cC8h"):
        m[k] = consts[k]
    return m


_NC_CACHE = {}


def kernel(**inputs):
    ins = {k: np.asarray(v) for k, v in inputs.items()}
    if "nc" not in _NC_CACHE:
        _NC_CACHE["nc"] = build_nc()
    nc = _NC_CACHE["nc"]
    consts = _host_consts()
    in_maps = [_prep_core_inputs(c, ins, consts) for c in range(N_CORES)]
    res = run_bass_kernel_spmd(nc, in_maps, core_ids=list(range(N_CORES)))
    out = np.zeros((N, LQ, DM), np.float32)
    for c in range(N_CORES):
        b, qh = c // 2, c % 2
        out[b, qh * QPC:(qh + 1) * QPC] = res.results[c]["out"]
    return out

